# revision 1
# baseline (speedup 1.0000x reference)
"""BlurPool3D Trainium2 kernel (~152 us on 8 cores; DMA roofline ~121 us).

Depthwise 3x3x3 separable (rank-1) blur, stride 2, pad 1 on
x[2, 64, 64, 96, 96] f32 -> y[2, 64, 32, 48, 48]. All arithmetic fp32
(rel err vs fp32 reference ~8e-8).

Strategy (8 NeuronCores, SPMD, fully DMA/compute overlapped):
  - Shard the 128 (n, c) pairs across cores: 16 per core. Channels are
    independent in a depthwise conv -> no halo, no collectives.
  - Per core, 8 blocks of 2 channels. SBUF layout per block:
    partitions = (2 nc x 64 d) = 128, free = spatial. The full D axis
    lives on partitions, so the D-tap contraction is a single matmul
    with a block-diagonal band lhsT [128, 64] (d' columns) -- D edges
    handled by the matrix, no halo exchange anywhere.
  - W-pass on VectorE: 3-tap stride-2 blur along W as two fused
    scalar_tensor_tensor ops per row range (out = (mid*r1) + left;
    out = (right*r2) + out), emitted per chunk-aligned piece tile
    (29/21 h-rows per h-half) so every matmul depends on one short
    DMA -> STT chain.
  - H handling is split to balance engines: the first XH=4 output rows
    of each 24-row h-half are H-blurred on VectorE (2 more STTs), then
    need only a 1-tap D-only matmul; the remaining rows use a 3-tap
    fused H+D matmul (H shift = strided rhs access pattern, H taps
    folded into 3 band-matrix variants).
  - The two h-halves map to PE column groups 0/1 (tile_position
    (0,0)/(0,64)) writing PSUM partitions 0-63/64-127; matmuls are
    emitted g-major so a half's taps never head-block the other's in
    the PE FIFO. ScalarE drains PSUM -> SBUF [128, 1152].
  - Input DMAs ride both HWDGE rings (SP + ACT); output DMAs go per
    h-half on the ACT ring (partition halves hit disjoint SBUF ports).
"""

import os
import sys

for _p in ("/opt/trn_rl_repo",):
    if _p not in sys.path and os.path.isdir(_p):
        sys.path.insert(0, _p)

import numpy as np

N, C, D, H, W = 2, 64, 64, 96, 96
DO, HO, WO = 32, 48, 48
NCORES = 8
NC_PER_CORE = (N * C) // NCORES  # 16
BLOCKS = NC_PER_CORE // 2  # 8 blocks of 2 channels each
HP = H + 1  # h-padded rows in the W-blurred tile

_PROGRAM_CACHE = {}


def _rank1_factors(filt):
    """Per-channel rank-1 factorization filt[c,0] = outer(d, h, w).

    Returns (dvec, hvec, wvec) each [C, 3] with
    filt[c, 0, i, j, k] == dvec[c,i] * hvec[c,j] * wvec[c,k].
    Exact for true rank-1 filters (e.g. the binomial blur, whose entries
    are all powers of two).
    """
    dvec = np.empty((C, 3), np.float64)
    hvec = np.empty((C, 3), np.float64)
    wvec = np.empty((C, 3), np.float64)
    for c in range(C):
        T = filt[c, 0].astype(np.float64)
        idx = np.unravel_index(np.argmax(np.abs(T)), T.shape)
        i0, j0, k0 = idx
        piv = T[i0, j0, k0]
        if piv == 0.0:
            # all-zero filter
            dvec[c] = hvec[c] = wvec[c] = 0.0
            continue
        dvec[c] = T[:, j0, k0]
        hvec[c] = T[i0, :, k0] / piv
        wvec[c] = T[i0, j0, :] / piv
        recon = np.einsum("i,j,k->ijk", dvec[c], hvec[c], wvec[c])
        resid = np.abs(recon - T).max()
        if resid > 1e-6 * max(np.abs(T).max(), 1e-30):
            raise ValueError(f"filter channel {c} is not rank-1 (resid {resid})")
    return dvec, hvec, wvec


def _build_program(uniform):
    import concourse.bacc as bacc
    import concourse.mybir as mybir
    from concourse import tile

    dt = mybir.dt
    nc = bacc.Bacc("TRN2", target_bir_lowering=False, debug=False,
                   num_devices=NCORES)

    nbm = 1 if uniform else BLOCKS
    x = nc.dram_tensor("x", [NC_PER_CORE, D, H * W], dt.float32,
                       kind="ExternalInput")
    bmat = nc.dram_tensor("bmat", [128, nbm * 4 * 64], dt.float32,
                          kind="ExternalInput")
    wtaps = nc.dram_tensor("wtaps", [128, 4 * BLOCKS], dt.float32,
                           kind="ExternalInput")
    y = nc.dram_tensor("y", [NC_PER_CORE, DO, HO * WO], dt.float32,
                       kind="ExternalOutput")

    # 48 output h-rows per block: two halves of 24 mapped to PE
    # column-groups 0/1 (PSUM partitions 0-63 / 64-127). Rows 0-3 and
    # 14-23 of each half are H-blurred on VectorE (then a single D-only
    # matmul); rows 4-13 use the 3-tap fused H+D matmul. This keeps the
    # expensive 3-tap matmuls to one 480-col chunk per half per block.
    HHALF = 24
    XH = 4            # DVE-H rows (0..3) from piece p0
    NHT = XH
    hd_chunks = [(4, 10), (14, 10)]
    d_chunks = [(0, 4)]             # (h0, cnt); ht rows below
    ht_rows = {0: (0, 4)}

    with tile.TileContext(nc) as tc:
        with tc.tile_pool(name="const", bufs=1) as cpool, \
             tc.tile_pool(name="xp", bufs=6) as xpool, \
             tc.tile_pool(name="wp", bufs=6) as wpool, \
             tc.tile_pool(name="hp", bufs=4) as hpool, \
             tc.tile_pool(name="op", bufs=4) as opool, \
             tc.tile_pool(name="ps", bufs=8, space="PSUM") as pspool:
            bt = cpool.tile([128, nbm * 4 * 64], dt.float32)
            wt = cpool.tile([128, 4 * BLOCKS], dt.float32)
            nc.sync.dma_start(bt[:], bmat[:])
            nc.sync.dma_start(wt[:], wtaps[:])

            for b in range(BLOCKS):
                bcol = 0 if uniform else b * 4 * 64
                r1 = wt[:, 4 * b:4 * b + 1]
                r2 = wt[:, 4 * b + 1:4 * b + 2]
                hr1 = wt[:, 4 * b + 2:4 * b + 3]
                hr2 = wt[:, 4 * b + 3:4 * b + 4]
                src = x[2 * b:2 * b + 2].rearrange("a d f -> (a d) f")
                src = src.rearrange("p (h w) -> p h w", h=H)

                def wpass(out_rows, xin):
                    # out = left + r1*mid + r2*right (w' = 1..47)
                    nc.vector.scalar_tensor_tensor(
                        out_rows[:, :, 1:WO],
                        xin[:, :, 2:2 * WO - 1:2], r1,
                        xin[:, :, 1:2 * WO - 2:2],
                        mybir.AluOpType.mult, mybir.AluOpType.add)
                    nc.vector.scalar_tensor_tensor(
                        out_rows[:, :, 1:WO],
                        xin[:, :, 3:2 * WO:2], r2, out_rows[:, :, 1:WO],
                        mybir.AluOpType.mult, mybir.AluOpType.add)
                    # w' = 0 edge (left tap is zero-pad)
                    nc.vector.tensor_scalar(
                        out_rows[:, :, 0:1], xin[:, :, 0:1], r1, None,
                        mybir.AluOpType.mult)
                    nc.vector.scalar_tensor_tensor(
                        out_rows[:, :, 0:1],
                        xin[:, :, 1:2], r2, out_rows[:, :, 0:1],
                        mybir.AluOpType.mult, mybir.AluOpType.add)

                # Each h-half (g) is loaded as two x sub-tiles (28 + 20
                # rows) and W-blurred into two piece tiles (29 + 21 rows)
                # that align exactly with the PSUM chunks below, so each
                # matmul depends on one short DMA -> STT chain. Piece p0
                # row j = W-blur(x row 48g + j - 1), p1 row j = W-blur(x
                # row 48g + 27 + j); boundary rows are recomputed, not
                # re-DMAed.
                xt0s, xt1s, pieces = [], [], {}
                for g in range(2):
                    xt0 = xpool.tile([128, 28, W], dt.float32, tag="xt0")
                    xt1 = xpool.tile([128, 20, W], dt.float32, tag="xt1")
                    nc.sync.dma_start(xt0[:], src[:, 48 * g:48 * g + 28, :])
                    nc.scalar.dma_start(xt1[:], src[:, 48 * g + 28:
                                                    48 * (g + 1), :])
                    xt0s.append(xt0)
                    xt1s.append(xt1)
                ht = hpool.tile([128, 2, NHT, WO], dt.float32, name="ht")
                for g in range(2):
                    p0 = wpool.tile([128, 29, WO], dt.float32, tag="p0")
                    p1 = wpool.tile([128, 21, WO], dt.float32, tag="p1")
                    pieces[(g, 0)] = p0
                    pieces[(g, 1)] = p1
                    if g == 0:
                        nc.gpsimd.memset(p0[:, 0, :], 0.0)
                    else:
                        wpass(p0[:, 0:1, :], xt1s[0][:, 19:20, :])
                    wpass(p0[:, 1:29, :], xt0s[g][:, :, :])
                    # H pass rows 0..3 right after the p0 piece
                    nc.vector.scalar_tensor_tensor(
                        ht[:, g, 0:XH, :], p0[:, 1:2 * XH:2, :], hr1,
                        p0[:, 0:2 * XH - 1:2, :],
                        mybir.AluOpType.mult, mybir.AluOpType.add)
                    nc.vector.scalar_tensor_tensor(
                        ht[:, g, 0:XH, :], p0[:, 2:2 * XH + 1:2, :], hr2,
                        ht[:, g, 0:XH, :],
                        mybir.AluOpType.mult, mybir.AluOpType.add)
                    wpass(p1[:, 0:1, :], xt0s[g][:, 27:28, :])
                    wpass(p1[:, 1:21, :], xt1s[g][:, :, :])

                # ---- D(-only) / fused H+D matmuls + PSUM drain ----
                # out tile partitions: (h-half, ncl, d'); per-partition free
                # run = 24h' x 48w = 1152 contiguous output elements
                ot = opool.tile([128, HHALF * WO], dt.float32)
                # g-major emission: PE can run a whole half's taps as
                # soon as that half's piece is ready (no FIFO head-block
                # on the other half)
                pss = {}
                for h0, cnt in hd_chunks + d_chunks:
                    pss[h0] = pspool.tile([128, 10 * WO], dt.float32,
                                          tag="ps", name="ps")
                for g in range(2):
                    # order by data readiness: p0-dependent first, then
                    # the ht chunk (also p0-derived), then p1-dependent
                    for h0, cnt in [hd_chunks[0]]:
                        psv = pss[h0][:, :cnt * WO]
                        pi = 0 if h0 < 14 else 1
                        roff = 2 * h0 - 28 * pi
                        for k in range(3):
                            lhsT = bt[:, bcol + k * 64:bcol + (k + 1) * 64]
                            rhs = pieces[(g, pi)][:, roff + k:
                                                  roff + k + 2 * cnt - 1:2, :]
                            nc.tensor.matmul(
                                psv[g * 64:, :] if g else psv[:64, :],
                                lhsT, rhs,
                                start=(k == 0), stop=(k == 2),
                                tile_position=(0, 64 * g) if g else None)
                    for h0, cnt in d_chunks:
                        psv = pss[h0][:, :cnt * WO]
                        lhsT = bt[:, bcol + 3 * 64:bcol + 4 * 64]
                        ra, rb = ht_rows[h0]
                        rhs = ht[:, g, ra:rb, :]
                        nc.tensor.matmul(
                            psv[g * 64:, :] if g else psv[:64, :],
                            lhsT, rhs, start=True, stop=True,
                            tile_position=(0, 64 * g) if g else None)
                    for h0, cnt in hd_chunks[1:]:
                        psv = pss[h0][:, :cnt * WO]
                        pi = 0 if h0 < 14 else 1
                        roff = 2 * h0 - 28 * pi
                        for k in range(3):
                            lhsT = bt[:, bcol + k * 64:bcol + (k + 1) * 64]
                            rhs = pieces[(g, pi)][:, roff + k:
                                                  roff + k + 2 * cnt - 1:2, :]
                            nc.tensor.matmul(
                                psv[g * 64:, :] if g else psv[:64, :],
                                lhsT, rhs,
                                start=(k == 0), stop=(k == 2),
                                tile_position=(0, 64 * g) if g else None)
                for h0, cnt in hd_chunks + d_chunks:
                    nc.scalar.copy(ot[:, h0 * WO:(h0 + cnt) * WO],
                                   pss[h0][:, :cnt * WO])

                # one DMA per h-half on the two HWDGE rings (SP / ACT) —
                # they move disjoint partition halves via disjoint SBUF
                # ports, so they run in parallel
                for g, eng in ((0, nc.scalar), (1, nc.scalar)):
                    dst = y[2 * b:2 * b + 2, :, g * HHALF * WO:
                            (g + 1) * HHALF * WO]
                    dst = dst.rearrange("a d f -> (a d) f")
                    eng.dma_start(dst, ot[g * 64:(g + 1) * 64, :])
    nc.compile()
    return nc


def kernel(x, filt):
    x = np.ascontiguousarray(np.asarray(x, dtype=np.float32))
    filt = np.asarray(filt, dtype=np.float32)
    assert x.shape == (N, C, D, H, W), x.shape

    from concourse.bass_utils import run_bass_kernel_spmd

    dvec, hvec, wvec = _rank1_factors(filt)
    # W/H pivots (left taps w0/h0); both folded into the matmul matrices.
    w0 = wvec[:, 0].copy()
    h0v = hvec[:, 0].copy()
    safe = (np.abs(w0) > 1e-30) & (np.abs(h0v) > 1e-30)
    if not safe.all():
        raise ValueError("W/H-tap pivot is zero; unsupported filter")
    r1 = wvec[:, 1] / w0
    r2 = wvec[:, 2] / w0
    hr1 = hvec[:, 1] / h0v
    hr2 = hvec[:, 2] / h0v

    uniform = bool(np.all(filt == filt[:1]))
    xr = x.reshape(N * C, D, H * W)

    in_maps = []
    for core in range(NCORES):
        chans = (np.arange(NC_PER_CORE) + core * NC_PER_CORE) % C  # local->c
        # wtaps[p, 4b+j]: partition p = (ncl, d); channel = chans[2b + ncl]
        wt = np.empty((128, 4 * BLOCKS), np.float32)
        bm = np.zeros((128, (1 if uniform else BLOCKS) * 4 * 64), np.float32)
        for b in range(BLOCKS):
            for ncl in range(2):
                c = chans[2 * b + ncl]
                wt[ncl * 64:(ncl + 1) * 64, 4 * b + 0] = r1[c]
                wt[ncl * 64:(ncl + 1) * 64, 4 * b + 1] = r2[c]
                wt[ncl * 64:(ncl + 1) * 64, 4 * b + 2] = hr1[c]
                wt[ncl * 64:(ncl + 1) * 64, 4 * b + 3] = hr2[c]
                if uniform and b > 0:
                    continue
                # band matrix rows (ncl*64 + d), cols (ncl*32 + d').
                # k = 0..2: fused H+D taps (x hvec[k]); k = 3: D-only
                # (x h0 pivot, pairing with the VectorE H pass).
                for k in range(4):
                    col0 = (b * 4 + k) * 64 + ncl * 32
                    hscale = hvec[c, k] if k < 3 else h0v[c]
                    for dp in range(DO):
                        for delta in range(3):
                            d = 2 * dp - 1 + delta
                            if 0 <= d < D:
                                bm[ncl * 64 + d, col0 + dp] = (
                                    dvec[c, delta] * hscale * w0[c])
        in_maps.append({
            "x": np.ascontiguousarray(
                xr[core * NC_PER_CORE:(core + 1) * NC_PER_CORE]),
            "bmat": bm,
            "wtaps": wt,
        })

    key = ("prog", uniform)
    if key not in _PROGRAM_CACHE:
        _PROGRAM_CACHE[key] = _build_program(uniform)
    nc = _PROGRAM_CACHE[key]

    trace = bool(int(os.environ.get("BLURPOOL_TRACE", "0")))
    kwargs = {}
    if trace and os.environ.get("BLURPOOL_TRACE_DIR"):
        kwargs["tmpdir"] = os.environ["BLURPOOL_TRACE_DIR"]
    res = run_bass_kernel_spmd(nc, in_maps, core_ids=list(range(NCORES)),
                               trace=trace, **kwargs)
    if trace:
        kernel.last_result = res

    out = np.concatenate([r["y"].reshape(NC_PER_CORE, DO, HO, WO)
                          for r in res.results], axis=0)
    return np.ascontiguousarray(out.reshape(N, C, DO, HO, WO))



# revision 3
# speedup vs baseline: 1.3432x; 1.3432x over previous
"""BlurPool3D Trainium2 kernel — bf16, DMA-roofline oriented.

Depthwise 3x3x3 separable (rank-1) blur, stride 2, pad 1 on
x[2, 64, 64, 96, 96] -> y[2, 64, 32, 48, 48].

The correctness gate is rel_err < 2e-2, which admits bf16 end-to-end:
input is cast to bf16 on the host (the binomial filter's taps and tap
ratios are exact powers of two in bf16), halving HBM traffic vs fp32.
Per-core traffic 16x(64*96*96 + 32*48*48)*2B = 21.2 MB -> ~62 us DMA
roofline at ~345 GB/s effective.

Engine split (per core: 16 (n,c) pairs = 8 blocks of 2 channels;
SBUF partitions = 2 nc x 64 d):
  - DVE does the H-blur: stride-2 over h-ROWS keeps the innermost
    element step at 1, so bf16 2x packing engages (the baseline's
    W-pass read stride-2 *elements*, which blocks packing and made
    DVE the 94%-busy bottleneck). 2 STTs per 24-row half, reading a
    97-row x tile whose row 0 is memset to zero (h=-1 pad).
  - TensorE does W and D fused: 3 accumulated matmuls per PSUM chunk,
    one per W tap, each with a block-diagonal D-band lhsT [128, 64]
    (bf16 = 4x the fp32 matmul rate) and an rhs that reads the
    H-blurred tile at w offset (k-1) with element stride 2 - strides
    are free in the moving-operand AP. The w'=0 left tap (k=0) is a
    partial matmul over w' 1..47; PSUM start on k=1, stop on k=2.
  - Two blocks share each PSUM tile via PE column groups 0/1
    (tile_position (0,0)/(0,64)), so drains and output DMAs run at
    full 128-partition width: ScalarE drains PSUM fp32 -> bf16 SBUF,
    one 590 KB output DMA per block pair.
"""

import os
import sys

for _p in ("/opt/trn_rl_repo",):
    if _p not in sys.path and os.path.isdir(_p):
        sys.path.insert(0, _p)

import ml_dtypes
import numpy as np

N, C, D, H, W = 2, 64, 64, 96, 96
DO, HO, WO = 32, 48, 48
NCORES = 8
NC_PER_CORE = (N * C) // NCORES  # 16
BLOCKS = NC_PER_CORE // 2  # 8 blocks of 2 channels each
PAIRS = BLOCKS // 2
XROWS = H + 1  # zero pad row + 96 x rows
# h'-row chunks: PSUM bank holds 512 fp32 -> up to 10 rows x 48 w'
CHUNKS = [(0, 10), (10, 10), (20, 10), (30, 10), (40, 8)]

BF16 = ml_dtypes.bfloat16

_PROGRAM_CACHE = {}


def _rank1_factors(filt):
    """Per-channel rank-1 factorization filt[c,0] = outer(d, h, w)."""
    dvec = np.empty((C, 3), np.float64)
    hvec = np.empty((C, 3), np.float64)
    wvec = np.empty((C, 3), np.float64)
    for c in range(C):
        T = filt[c, 0].astype(np.float64)
        idx = np.unravel_index(np.argmax(np.abs(T)), T.shape)
        i0, j0, k0 = idx
        piv = T[i0, j0, k0]
        if piv == 0.0:
            dvec[c] = hvec[c] = wvec[c] = 0.0
            continue
        dvec[c] = T[:, j0, k0]
        hvec[c] = T[i0, :, k0] / piv
        wvec[c] = T[i0, j0, :] / piv
        recon = np.einsum("i,j,k->ijk", dvec[c], hvec[c], wvec[c])
        resid = np.abs(recon - T).max()
        if resid > 1e-6 * max(np.abs(T).max(), 1e-30):
            raise ValueError(f"filter channel {c} is not rank-1 (resid {resid})")
    return dvec, hvec, wvec


def _build_program(uniform):
    import concourse.bacc as bacc
    import concourse.mybir as mybir
    from concourse import tile

    dt = mybir.dt
    bf = dt.bfloat16
    mult = mybir.AluOpType.mult
    add = mybir.AluOpType.add
    nc = bacc.Bacc("TRN2", target_bir_lowering=False, debug=False,
                   num_devices=NCORES)

    nbm = 1 if uniform else BLOCKS
    x = nc.dram_tensor("x", [NC_PER_CORE, D, H * W], bf,
                       kind="ExternalInput")
    bmat = nc.dram_tensor("bmat", [128, nbm * 3 * 64], bf,
                          kind="ExternalInput")
    wtaps = nc.dram_tensor("wtaps", [128, 2 * BLOCKS], bf,
                           kind="ExternalInput")
    y = nc.dram_tensor("y", [NC_PER_CORE, DO, HO * WO], bf,
                       kind="ExternalOutput")

    with tile.TileContext(nc) as tc:
        with tc.tile_pool(name="const", bufs=1) as cpool, \
             tc.tile_pool(name="xp", bufs=3) as xpool, \
             tc.tile_pool(name="hp", bufs=3) as hpool, \
             tc.tile_pool(name="op", bufs=2) as opool, \
             tc.tile_pool(name="ps", bufs=8, space="PSUM") as pspool:
            bt = cpool.tile([128, nbm * 3 * 64], bf)
            wt = cpool.tile([128, 2 * BLOCKS], bf)
            nc.sync.dma_start(bt[:], bmat[:])
            nc.sync.dma_start(wt[:], wtaps[:])

            hbs = {}
            for bp in range(PAIRS):
                for q in range(2):
                    b = 2 * bp + q
                    hr1 = wt[:, 2 * b:2 * b + 1]
                    hr2 = wt[:, 2 * b + 1:2 * b + 2]
                    src = x[2 * b:2 * b + 2].rearrange("a d f -> (a d) f")
                    src = src.rearrange("p (h w) -> p h w", h=H)
                    xt = xpool.tile([128, XROWS, W], bf, tag="xt")
                    nc.gpsimd.memset(xt[:, 0, :], 0.0)
                    nc.sync.dma_start(xt[:, 1:49, :], src[:, 0:48, :])
                    nc.scalar.dma_start(xt[:, 49:97, :], src[:, 48:96, :])
                    hb = hpool.tile([128, HO, W], bf, tag="hb")
                    hbs[b] = hb
                    for g in range(2):
                        rows = hb[:, 24 * g:24 * g + 24, :]
                        base = 48 * g
                        # out rows h' in [24g, 24g+24): taps are xt rows
                        # 2h', 2h'+1, 2h'+2 (xt row r = x row r-1; row 0
                        # is the h=-1 zero pad)
                        nc.vector.scalar_tensor_tensor(
                            rows,
                            xt[:, base + 1:base + 48:2, :], hr1,
                            xt[:, base + 0:base + 47:2, :],
                            mult, add)
                        nc.vector.scalar_tensor_tensor(
                            rows,
                            xt[:, base + 2:base + 49:2, :], hr2,
                            rows,
                            mult, add)

                # ---- fused W+D matmuls, PE column group q per block ----
                pst = {}
                for h0, cnt in CHUNKS:
                    pst[h0] = pspool.tile([128, 10, WO], dt.float32,
                                          tag="ps", name="ps")
                for q in range(2):
                    b = 2 * bp + q
                    hb = hbs[b]
                    bcol = 0 if uniform else b * 3 * 64
                    for k in (1, 0, 2):
                        lhsT = bt[:, bcol + k * 64:bcol + (k + 1) * 64]
                        for h0, cnt in CHUNKS:
                            ps = pst[h0]
                            if k == 0:
                                # w = 2w'-1 exists only for w' >= 1
                                out = ps[64 * q:64 * q + 64, 0:cnt, 1:WO]
                                rhs = hb[:, h0:h0 + cnt, 1:2 * WO - 1:2]
                            else:
                                out = ps[64 * q:64 * q + 64, 0:cnt, :]
                                rhs = hb[:, h0:h0 + cnt, k - 1:W:2]
                            nc.tensor.matmul(
                                out, lhsT, rhs,
                                start=(k == 1), stop=(k == 2),
                                tile_position=(0, 64 * q) if q else None)

                ot = opool.tile([128, HO * WO], bf)
                ot3 = ot[:, :].rearrange("p (h w) -> p h w", h=HO)
                for h0, cnt in CHUNKS:
                    nc.scalar.copy(ot3[:, h0:h0 + cnt, :],
                                   pst[h0][:, 0:cnt, :])
                dst = y[4 * bp:4 * bp + 4].rearrange("a d f -> (a d) f")
                nc.scalar.dma_start(dst, ot[:, :])
    nc.compile()
    return nc


def kernel(x, filt):
    x = np.ascontiguousarray(np.asarray(x, dtype=np.float32))
    filt = np.asarray(filt, dtype=np.float32)
    assert x.shape == (N, C, D, H, W), x.shape

    from concourse.bass_utils import run_bass_kernel_spmd

    dvec, hvec, wvec = _rank1_factors(filt)
    h0v = hvec[:, 0].copy()
    if not (np.abs(h0v) > 1e-30).all():
        raise ValueError("H-tap pivot is zero; unsupported filter")
    hr1 = hvec[:, 1] / h0v
    hr2 = hvec[:, 2] / h0v

    uniform = bool(np.all(filt == filt[:1]))
    xb = x.reshape(N * C, D, H * W).astype(BF16)

    in_maps = []
    for core in range(NCORES):
        chans = (np.arange(NC_PER_CORE) + core * NC_PER_CORE) % C
        wtp = np.empty((128, 2 * BLOCKS), np.float32)
        bm = np.zeros((128, (1 if uniform else BLOCKS) * 3 * 64), np.float32)
        for b in range(BLOCKS):
            for ncl in range(2):
                c = chans[2 * b + ncl]
                wtp[ncl * 64:(ncl + 1) * 64, 2 * b + 0] = hr1[c]
                wtp[ncl * 64:(ncl + 1) * 64, 2 * b + 1] = hr2[c]
                if uniform and b > 0:
                    continue
                # band rows (ncl*64 + d), cols (ncl*32 + d'), one band
                # per W tap k; D taps live inside the band, scaled by
                # the W tap and the H pivot.
                for k in range(3):
                    col0 = (0 if uniform else b * 3 * 64) + k * 64 + ncl * 32
                    for dp in range(DO):
                        for delta in range(3):
                            d = 2 * dp - 1 + delta
                            if 0 <= d < D:
                                bm[ncl * 64 + d, col0 + dp] = (
                                    dvec[c, delta] * wvec[c, k] * h0v[c])
        in_maps.append({
            "x": np.ascontiguousarray(
                xb[core * NC_PER_CORE:(core + 1) * NC_PER_CORE]),
            "bmat": bm.astype(BF16),
            "wtaps": wtp.astype(BF16),
        })

    key = ("prog", uniform)
    if key not in _PROGRAM_CACHE:
        _PROGRAM_CACHE[key] = _build_program(uniform)
    nc = _PROGRAM_CACHE[key]

    trace = bool(int(os.environ.get("BLURPOOL_TRACE", "0")))
    kwargs = {}
    if trace and os.environ.get("BLURPOOL_TRACE_DIR"):
        kwargs["tmpdir"] = os.environ["BLURPOOL_TRACE_DIR"]
    res = run_bass_kernel_spmd(nc, in_maps, core_ids=list(range(NCORES)),
                               trace=trace, **kwargs)
    if trace:
        kernel.last_result = res

    out = np.concatenate([np.asarray(r["y"]).astype(np.float32)
                          .reshape(NC_PER_CORE, DO, HO, WO)
                          for r in res.results], axis=0)
    return np.ascontiguousarray(out.reshape(N, C, DO, HO, WO))


# revision 9
# speedup vs baseline: 1.4824x; 1.1037x over previous
"""BlurPool3D Trainium2 kernel — bf16, DMA-roofline oriented.

Depthwise 3x3x3 separable (rank-1) blur, stride 2, pad 1 on
x[2, 64, 64, 96, 96] -> y[2, 64, 32, 48, 48].

The correctness gate is rel_err < 2e-2, which admits bf16 end-to-end:
input is cast to bf16 on the host (the binomial filter's taps and tap
ratios are exact powers of two in bf16), halving HBM traffic vs fp32.
Per-core traffic 16x(64*96*96 + 32*48*48)*2B = 21.2 MB -> ~62 us DMA
roofline at ~345 GB/s effective.

Engine split (per core: 16 (n,c) pairs = 8 blocks of 2 channels;
SBUF partitions = 2 nc x 64 d):
  - DVE does the H-blur: stride-2 over h-ROWS keeps the innermost
    element step at 1, so bf16 packing engages (the baseline's
    W-pass read stride-2 *elements*, which blocks packing and made
    DVE the 94%-busy bottleneck). scalar_tensor_tensor supports NO
    dve perf modes (measured 1 elem/cy), so the 3-tap blur is split
    into tensor_scalar (mid*r1, 4x) + tensor_tensor (top+bot, 2x) +
    in-place tensor_tensor add (2x), reading a 97-row x tile whose
    row 0 is memset to zero (h=-1 pad). The mid*r1 scale of one
    h-half runs on ScalarE (activation scale-copy) to balance
    engines. GpSimd TT is ~2.6 cyc/elem - not used for bulk.
  - TensorE does W and D fused: 3 accumulated matmuls per PSUM chunk,
    one per W tap, each with a block-diagonal D-band lhsT [128, 64]
    (bf16 = 4x the fp32 matmul rate) and an rhs that reads the
    H-blurred tile at w offset (k-1) with element stride 2 - strides
    are free in the moving-operand AP. The w'=0 left tap (k=0) is a
    partial matmul over w' 1..47; PSUM start on k=1, stop on k=2.
  - Two blocks share each PSUM tile via PE column groups 0/1
    (tile_position (0,0)/(0,64)), so drains and output DMAs run at
    full 128-partition width: ScalarE drains PSUM fp32 -> bf16 SBUF,
    one 590 KB output DMA per block pair.
"""

import os
import sys

for _p in ("/opt/trn_rl_repo",):
    if _p not in sys.path and os.path.isdir(_p):
        sys.path.insert(0, _p)

import ml_dtypes
import numpy as np

N, C, D, H, W = 2, 64, 64, 96, 96
DO, HO, WO = 32, 48, 48
NCORES = 8
NC_PER_CORE = (N * C) // NCORES  # 16
BLOCKS = NC_PER_CORE // 2  # 8 blocks of 2 channels each
PAIRS = BLOCKS // 2
XROWS = H + 1  # zero pad row + 96 x rows
# h'-row chunks: PSUM bank holds 512 fp32 -> up to 10 rows x 48 w'
CHUNKS = [(0, 10), (10, 10), (20, 10), (30, 10), (40, 8)]

BF16 = ml_dtypes.bfloat16

_PROGRAM_CACHE = {}


def _rank1_factors(filt):
    """Per-channel rank-1 factorization filt[c,0] = outer(d, h, w)."""
    dvec = np.empty((C, 3), np.float64)
    hvec = np.empty((C, 3), np.float64)
    wvec = np.empty((C, 3), np.float64)
    for c in range(C):
        T = filt[c, 0].astype(np.float64)
        idx = np.unravel_index(np.argmax(np.abs(T)), T.shape)
        i0, j0, k0 = idx
        piv = T[i0, j0, k0]
        if piv == 0.0:
            dvec[c] = hvec[c] = wvec[c] = 0.0
            continue
        dvec[c] = T[:, j0, k0]
        hvec[c] = T[i0, :, k0] / piv
        wvec[c] = T[i0, j0, :] / piv
        recon = np.einsum("i,j,k->ijk", dvec[c], hvec[c], wvec[c])
        resid = np.abs(recon - T).max()
        if resid > 1e-6 * max(np.abs(T).max(), 1e-30):
            raise ValueError(f"filter channel {c} is not rank-1 (resid {resid})")
    return dvec, hvec, wvec


def _build_program(uniform, r2_one):
    import concourse.bacc as bacc
    import concourse.mybir as mybir
    from concourse import tile

    dt = mybir.dt
    bf = dt.bfloat16
    add = mybir.AluOpType.add
    nc = bacc.Bacc("TRN2", target_bir_lowering=False, debug=False,
                   num_devices=NCORES)

    nbm = 1 if uniform else BLOCKS
    x = nc.dram_tensor("x", [NC_PER_CORE, D, H * W], bf,
                       kind="ExternalInput")
    bmat = nc.dram_tensor("bmat", [128, nbm * 3 * 64], bf,
                          kind="ExternalInput")
    wtaps = nc.dram_tensor("wtaps", [128, 2 * BLOCKS], dt.float32,
                           kind="ExternalInput")
    y = nc.dram_tensor("y", [NC_PER_CORE, DO, HO * WO], bf,
                       kind="ExternalOutput")

    with tile.TileContext(nc) as tc:
        with tc.tile_pool(name="const", bufs=1) as cpool, \
             tc.tile_pool(name="xp", bufs=3) as xpool, \
             tc.tile_pool(name="hp", bufs=3) as hpool, \
             tc.tile_pool(name="mp", bufs=3) as mpool, \
             tc.tile_pool(name="op", bufs=2) as opool, \
             tc.tile_pool(name="ps", bufs=8, space="PSUM") as pspool:
            bt = cpool.tile([128, nbm * 3 * 64], bf)
            wt = cpool.tile([128, 2 * BLOCKS], dt.float32)
            nc.sync.dma_start(bt[:], bmat[:])
            nc.sync.dma_start(wt[:], wtaps[:])

            hbs = {}
            for bp in range(PAIRS):
                for q in range(2):
                    b = 2 * bp + q
                    hr1 = wt[:, 2 * b:2 * b + 1]
                    hr2 = wt[:, 2 * b + 1:2 * b + 2]
                    src = x[2 * b:2 * b + 2].rearrange("a d f -> (a d) f")
                    src = src.rearrange("p (h w) -> p h w", h=H)
                    xt = xpool.tile([128, XROWS, W], bf, tag="xt")
                    nc.gpsimd.memset(xt[:, 0, :], 0.0)
                    nc.sync.dma_start(xt[:, 1:49, :], src[:, 0:48, :])
                    nc.scalar.dma_start(xt[:, 49:97, :], src[:, 48:96, :])
                    hb = hpool.tile([128, HO, W], bf, tag="hb")
                    hbs[b] = hb
                    for g in range(2):
                        rows = hb[:, 24 * g:24 * g + 24, :]
                        base = 48 * g
                        # out rows h' in [24g, 24g+24): taps are xt rows
                        # 2h', 2h'+1, 2h'+2 (xt row r = x row r-1; row 0
                        # is the h=-1 zero pad)
                        top = xt[:, base + 0:base + 47:2, :]
                        mid = xt[:, base + 1:base + 48:2, :]
                        bot = xt[:, base + 2:base + 49:2, :]
                        m = mpool.tile([128, 24, W], bf, tag="m")
                        if g == 0:
                            # ScalarE: m = mid * r1 (engine balance)
                            nc.scalar.mul(m[:, :, :], mid, hr1)
                        else:
                            # DVE tensor_scalar packs 4x
                            nc.vector.tensor_scalar_mul(m[:, :, :], mid, hr1)
                        if r2_one:
                            nc.vector.tensor_tensor(rows, top, bot, add)
                        else:
                            nc.vector.tensor_scalar_mul(rows, bot, hr2)
                            nc.vector.tensor_tensor(rows, top, rows, add)
                        nc.vector.tensor_tensor(rows, m[:, :, :], rows, add)

                # ---- fused W+D matmuls, PE column group q per block ----
                pst = {}
                for h0, cnt in CHUNKS:
                    pst[h0] = pspool.tile([128, 10, WO], dt.float32,
                                          tag="ps", name="ps")
                for q in range(2):
                    b = 2 * bp + q
                    hb = hbs[b]
                    bcol = 0 if uniform else b * 3 * 64
                    for k in (1, 0, 2):
                        lhsT = bt[:, bcol + k * 64:bcol + (k + 1) * 64]
                        for h0, cnt in CHUNKS:
                            ps = pst[h0]
                            if k == 0:
                                # w = 2w'-1 exists only for w' >= 1
                                out = ps[64 * q:64 * q + 64, 0:cnt, 1:WO]
                                rhs = hb[:, h0:h0 + cnt, 1:2 * WO - 1:2]
                            else:
                                out = ps[64 * q:64 * q + 64, 0:cnt, :]
                                rhs = hb[:, h0:h0 + cnt, k - 1:W:2]
                            nc.tensor.matmul(
                                out, lhsT, rhs,
                                start=(k == 1), stop=(k == 2),
                                tile_position=(0, 64 * q) if q else None)

                ot = opool.tile([128, HO * WO], bf)
                ot3 = ot[:, :].rearrange("p (h w) -> p h w", h=HO)
                for h0, cnt in CHUNKS:
                    nc.scalar.copy(ot3[:, h0:h0 + cnt, :],
                                   pst[h0][:, 0:cnt, :])
                dst = y[4 * bp:4 * bp + 4].rearrange("a d f -> (a d) f")
                nc.scalar.dma_start(dst, ot[:, :])
    nc.compile()
    return nc


def kernel(x, filt):
    x = np.ascontiguousarray(np.asarray(x, dtype=np.float32))
    filt = np.asarray(filt, dtype=np.float32)
    assert x.shape == (N, C, D, H, W), x.shape

    from concourse.bass_utils import run_bass_kernel_spmd

    dvec, hvec, wvec = _rank1_factors(filt)
    h0v = hvec[:, 0].copy()
    if not (np.abs(h0v) > 1e-30).all():
        raise ValueError("H-tap pivot is zero; unsupported filter")
    hr1 = hvec[:, 1] / h0v
    hr2 = hvec[:, 2] / h0v

    uniform = bool(np.all(filt == filt[:1]))
    xb = x.reshape(N * C, D, H * W).astype(BF16)

    in_maps = []
    for core in range(NCORES):
        chans = (np.arange(NC_PER_CORE) + core * NC_PER_CORE) % C
        wtp = np.empty((128, 2 * BLOCKS), np.float32)
        bm = np.zeros((128, (1 if uniform else BLOCKS) * 3 * 64), np.float32)
        for b in range(BLOCKS):
            for ncl in range(2):
                c = chans[2 * b + ncl]
                wtp[ncl * 64:(ncl + 1) * 64, 2 * b + 0] = hr1[c]
                wtp[ncl * 64:(ncl + 1) * 64, 2 * b + 1] = hr2[c]
                if uniform and b > 0:
                    continue
                # band rows (ncl*64 + d), cols (ncl*32 + d'), one band
                # per W tap k; D taps live inside the band, scaled by
                # the W tap and the H pivot.
                for k in range(3):
                    col0 = (0 if uniform else b * 3 * 64) + k * 64 + ncl * 32
                    for dp in range(DO):
                        for delta in range(3):
                            d = 2 * dp - 1 + delta
                            if 0 <= d < D:
                                bm[ncl * 64 + d, col0 + dp] = (
                                    dvec[c, delta] * wvec[c, k] * h0v[c])
        in_maps.append({
            "x": np.ascontiguousarray(
                xb[core * NC_PER_CORE:(core + 1) * NC_PER_CORE]),
            "bmat": bm.astype(BF16),
            "wtaps": wtp,
        })

    r2_one = bool(np.all(hr2 == 1.0))
    key = ("prog", uniform, r2_one)
    if key not in _PROGRAM_CACHE:
        _PROGRAM_CACHE[key] = _build_program(uniform, r2_one)
    nc = _PROGRAM_CACHE[key]

    trace = bool(int(os.environ.get("BLURPOOL_TRACE", "0")))
    kwargs = {}
    if trace and os.environ.get("BLURPOOL_TRACE_DIR"):
        kwargs["tmpdir"] = os.environ["BLURPOOL_TRACE_DIR"]
    res = run_bass_kernel_spmd(nc, in_maps, core_ids=list(range(NCORES)),
                               trace=trace, **kwargs)
    if trace:
        kernel.last_result = res

    out = np.concatenate([np.asarray(r["y"]).astype(np.float32)
                          .reshape(NC_PER_CORE, DO, HO, WO)
                          for r in res.results], axis=0)
    return np.ascontiguousarray(out.reshape(N, C, DO, HO, WO))


# revision 13
# speedup vs baseline: 1.6764x; 1.1309x over previous
"""BlurPool3D Trainium2 kernel — bf16, DMA-roofline oriented.

Depthwise 3x3x3 separable (rank-1) blur, stride 2, pad 1 on
x[2, 64, 64, 96, 96] -> y[2, 64, 32, 48, 48].

The correctness gate is rel_err < 2e-2, which admits bf16 end-to-end:
input is cast to bf16 on the host (the binomial filter's taps and tap
ratios are exact powers of two in bf16), halving HBM traffic vs fp32.
Per-core traffic 16x(64*96*96 + 32*48*48)*2B = 21.2 MB -> ~60 us DMA
roofline at ~360 GB/s effective.

Engine split (per core: 16 (n,c) pairs = 8 blocks of 2 channels;
SBUF partitions = 2 nc x 64 d):
  - DVE does the H-blur: stride-2 over h-ROWS keeps the innermost
    element step at 1, so bf16 packing engages (a stride-2 *element*
    read, as in a direct W-pass, blocks packing). scalar_tensor_tensor
    supports NO dve perf modes (measured 1 elem/cy), so the 3-tap blur
    is tensor_scalar (mid*r1, 4x) + tensor_tensor (top+bot, 2x) +
    in-place tensor_tensor add (2x), on a 97-row x tile whose row 0 is
    memset to zero (h=-1 pad). The mid*r1 of the g=0 half runs on
    ScalarE (activation scale-copy) for engine balance.
  - TensorE does W and D fused: per h-half, 3 accumulated matmuls per
    PSUM chunk, one per W tap, with a block-diagonal D-band lhsT
    [128, 64] and rhs reading the H-blurred tile at w offset (k-1)
    with element stride 2 (strides are free in the moving-operand
    AP). The w'=0 left tap (k=0) is a partial matmul over w' 1..47.
  - The two 24-row h-halves of ONE block map to PE column groups 0/1
    (tile_position (0,0)/(0,64)) writing PSUM partitions 0-63/64-127,
    so matmul+drain+output run at block granularity (short pipeline
    tail) and full 128-partition width. ScalarE drains PSUM fp32 ->
    bf16 SBUF; one 295 KB output DMA per block.
  - Input DMAs alternate between the SP and ACT HWDGE rings; the
    first block's input is split into 4 quarter-DMAs so the first
    H-blur starts ~4 us earlier.
"""

import os
import sys

for _p in ("/opt/trn_rl_repo",):
    if _p not in sys.path and os.path.isdir(_p):
        sys.path.insert(0, _p)

import ml_dtypes
import numpy as np

N, C, D, H, W = 2, 64, 64, 96, 96
DO, HO, WO = 32, 48, 48
NCORES = 8
NC_PER_CORE = (N * C) // NCORES  # 16
BLOCKS = NC_PER_CORE // 2  # 8 blocks of 2 channels each
XROWS = H + 1  # zero pad row + 96 x rows
HH = HO // 2  # 24 output rows per h-half
# h''-row chunks within a 24-row half: PSUM bank holds 512 fp32
CHUNKS = [(0, 10), (10, 10), (20, 4)]

BF16 = ml_dtypes.bfloat16

_PROGRAM_CACHE = {}


def _rank1_factors(filt):
    """Per-channel rank-1 factorization filt[c,0] = outer(d, h, w)."""
    dvec = np.empty((C, 3), np.float64)
    hvec = np.empty((C, 3), np.float64)
    wvec = np.empty((C, 3), np.float64)
    for c in range(C):
        T = filt[c, 0].astype(np.float64)
        idx = np.unravel_index(np.argmax(np.abs(T)), T.shape)
        i0, j0, k0 = idx
        piv = T[i0, j0, k0]
        if piv == 0.0:
            dvec[c] = hvec[c] = wvec[c] = 0.0
            continue
        dvec[c] = T[:, j0, k0]
        hvec[c] = T[i0, :, k0] / piv
        wvec[c] = T[i0, j0, :] / piv
        recon = np.einsum("i,j,k->ijk", dvec[c], hvec[c], wvec[c])
        resid = np.abs(recon - T).max()
        if resid > 1e-6 * max(np.abs(T).max(), 1e-30):
            raise ValueError(f"filter channel {c} is not rank-1 (resid {resid})")
    return dvec, hvec, wvec


def _build_program(uniform, r2_one):
    import concourse.bacc as bacc
    import concourse.mybir as mybir
    from concourse import tile

    dt = mybir.dt
    bf = dt.bfloat16
    add = mybir.AluOpType.add
    nc = bacc.Bacc("TRN2", target_bir_lowering=False, debug=False,
                   num_devices=NCORES)

    nbm = 1 if uniform else BLOCKS
    x = nc.dram_tensor("x", [NC_PER_CORE, D, H * W], bf,
                       kind="ExternalInput")
    bmat = nc.dram_tensor("bmat", [128, nbm * 3 * 64], bf,
                          kind="ExternalInput")
    wtaps = nc.dram_tensor("wtaps", [128, 2 * BLOCKS], dt.float32,
                           kind="ExternalInput")
    # block-native layout [b, g(h-half), ncl, d', 24*48]; host permutes
    y = nc.dram_tensor("y", [BLOCKS, 2, 2, DO, HH * WO], bf,
                       kind="ExternalOutput")

    with tile.TileContext(nc) as tc:
        with tc.tile_pool(name="const", bufs=1) as cpool, \
             tc.tile_pool(name="xp", bufs=4) as xpool, \
             tc.tile_pool(name="hp", bufs=3) as hpool, \
             tc.tile_pool(name="mp", bufs=3) as mpool, \
             tc.tile_pool(name="op", bufs=3) as opool, \
             tc.tile_pool(name="ps", bufs=8, space="PSUM") as pspool:
            bt = cpool.tile([128, nbm * 3 * 64], bf)
            wt = cpool.tile([128, 2 * BLOCKS], dt.float32)
            nc.sync.dma_start(bt[:], bmat[:])
            nc.sync.dma_start(wt[:], wtaps[:])

            for b in range(BLOCKS):
                hr1 = wt[:, 2 * b:2 * b + 1]
                hr2 = wt[:, 2 * b + 1:2 * b + 2]
                src = x[2 * b:2 * b + 2].rearrange("a d f -> (a d) f")
                src = src.rearrange("p (h w) -> p h w", h=H)
                xt = xpool.tile([128, XROWS, W], bf, tag="xt")
                nc.gpsimd.memset(xt[:, 0, :], 0.0)
                if b == 0:
                    # quarter DMAs: the first H-blur half only needs
                    # rows 0..48, so compute starts ~4 us sooner
                    nc.sync.dma_start(xt[:, 1:25, :], src[:, 0:24, :])
                    nc.sync.dma_start(xt[:, 25:49, :], src[:, 24:48, :])
                    nc.scalar.dma_start(xt[:, 49:73, :], src[:, 48:72, :])
                    nc.scalar.dma_start(xt[:, 73:97, :], src[:, 72:96, :])
                else:
                    nc.sync.dma_start(xt[:, 1:49, :], src[:, 0:48, :])
                    nc.scalar.dma_start(xt[:, 49:97, :], src[:, 48:96, :])
                hb = hpool.tile([128, HO, W], bf, tag="hb")
                for g in range(2):
                    rows = hb[:, HH * g:HH * g + HH, :]
                    base = 48 * g
                    # out rows h' in [24g, 24g+24): taps are xt rows
                    # 2h', 2h'+1, 2h'+2 (xt row r = x row r-1; row 0
                    # is the h=-1 zero pad)
                    top = xt[:, base + 0:base + 47:2, :]
                    mid = xt[:, base + 1:base + 48:2, :]
                    bot = xt[:, base + 2:base + 49:2, :]
                    m = mpool.tile([128, HH, W], bf, tag="m")
                    if g == 0:
                        # ScalarE: m = mid * r1 (engine balance)
                        nc.scalar.mul(m[:, :, :], mid, hr1)
                    else:
                        # DVE tensor_scalar packs 4x
                        nc.vector.tensor_scalar_mul(m[:, :, :], mid, hr1)
                    if r2_one:
                        nc.vector.tensor_tensor(rows, top, bot, add)
                    else:
                        nc.vector.tensor_scalar_mul(rows, bot, hr2)
                        nc.vector.tensor_tensor(rows, top, rows, add)
                    nc.vector.tensor_tensor(rows, m[:, :, :], rows, add)

                # ---- fused W+D matmuls, PE column group g per h-half --
                bcol = 0 if uniform else b * 3 * 64
                pst = {}
                for h0, cnt in CHUNKS:
                    pst[h0] = pspool.tile([128, 10, WO], dt.float32,
                                          tag="ps", name="ps")
                for g in range(2):
                    for k in (1, 0, 2):
                        lhsT = bt[:, bcol + k * 64:bcol + (k + 1) * 64]
                        for h0, cnt in CHUNKS:
                            ps = pst[h0]
                            if k == 0:
                                # w = 2w'-1 exists only for w' >= 1
                                out = ps[64 * g:64 * g + 64, 0:cnt, 1:WO]
                                rhs = hb[:, HH * g + h0:HH * g + h0 + cnt,
                                         1:2 * WO - 1:2]
                            else:
                                out = ps[64 * g:64 * g + 64, 0:cnt, :]
                                rhs = hb[:, HH * g + h0:HH * g + h0 + cnt,
                                         k - 1:W:2]
                            nc.tensor.matmul(
                                out, lhsT, rhs,
                                start=(k == 1), stop=(k == 2),
                                tile_position=(0, 64 * g) if g else None)

                # out partitions (g, ncl, d'); free = 24 h'' x 48 w'
                ot = opool.tile([128, HH * WO], bf)
                ot3 = ot[:, :].rearrange("p (h w) -> p h w", h=HH)
                for h0, cnt in CHUNKS:
                    nc.scalar.copy(ot3[:, h0:h0 + cnt, :],
                                   pst[h0][:, 0:cnt, :])
                dst = y[b].rearrange("g a d f -> (g a d) f")
                nc.scalar.dma_start(dst, ot[:, :])
    nc.compile()
    return nc


def kernel(x, filt):
    x = np.ascontiguousarray(np.asarray(x, dtype=np.float32))
    filt = np.asarray(filt, dtype=np.float32)
    assert x.shape == (N, C, D, H, W), x.shape

    from concourse.bass_utils import run_bass_kernel_spmd

    dvec, hvec, wvec = _rank1_factors(filt)
    h0v = hvec[:, 0].copy()
    if not (np.abs(h0v) > 1e-30).all():
        raise ValueError("H-tap pivot is zero; unsupported filter")
    hr1 = hvec[:, 1] / h0v
    hr2 = hvec[:, 2] / h0v

    uniform = bool(np.all(filt == filt[:1]))
    xb = x.reshape(N * C, D, H * W).astype(BF16)

    in_maps = []
    for core in range(NCORES):
        chans = (np.arange(NC_PER_CORE) + core * NC_PER_CORE) % C
        wtp = np.empty((128, 2 * BLOCKS), np.float32)
        bm = np.zeros((128, (1 if uniform else BLOCKS) * 3 * 64), np.float32)
        for b in range(BLOCKS):
            for ncl in range(2):
                c = chans[2 * b + ncl]
                wtp[ncl * 64:(ncl + 1) * 64, 2 * b + 0] = hr1[c]
                wtp[ncl * 64:(ncl + 1) * 64, 2 * b + 1] = hr2[c]
                if uniform and b > 0:
                    continue
                # band rows (ncl*64 + d), cols (ncl*32 + d'), one band
                # per W tap k; D taps live inside the band, scaled by
                # the W tap and the H pivot.
                for k in range(3):
                    col0 = (0 if uniform else b * 3 * 64) + k * 64 + ncl * 32
                    for dp in range(DO):
                        for delta in range(3):
                            d = 2 * dp - 1 + delta
                            if 0 <= d < D:
                                bm[ncl * 64 + d, col0 + dp] = (
                                    dvec[c, delta] * wvec[c, k] * h0v[c])
        in_maps.append({
            "x": np.ascontiguousarray(
                xb[core * NC_PER_CORE:(core + 1) * NC_PER_CORE]),
            "bmat": bm.astype(BF16),
            "wtaps": wtp,
        })

    r2_one = bool(np.all(hr2 == 1.0))
    key = ("prog", uniform, r2_one)
    if key not in _PROGRAM_CACHE:
        _PROGRAM_CACHE[key] = _build_program(uniform, r2_one)
    nc = _PROGRAM_CACHE[key]

    trace = bool(int(os.environ.get("BLURPOOL_TRACE", "0")))
    kwargs = {}
    if trace and os.environ.get("BLURPOOL_TRACE_DIR"):
        kwargs["tmpdir"] = os.environ["BLURPOOL_TRACE_DIR"]
    res = run_bass_kernel_spmd(nc, in_maps, core_ids=list(range(NCORES)),
                               trace=trace, **kwargs)
    if trace:
        kernel.last_result = res

    parts = []
    for r in res.results:
        # [b, g, ncl, d', 24*48] -> [2b+ncl, d', 24g+h'', w']
        arr = np.asarray(r["y"]).reshape(BLOCKS, 2, 2, DO, HH, WO)
        arr = arr.transpose(0, 2, 3, 1, 4, 5)  # b, ncl, d', g, h'', w'
        parts.append(arr.reshape(NC_PER_CORE, DO, HO, WO))
    out = np.concatenate(parts, axis=0).astype(np.float32)
    return np.ascontiguousarray(out.reshape(N, C, DO, HO, WO))
